# revision 35
# baseline (speedup 1.0000x reference)
"""ACE layer (moe_routing) Trainium2 kernel — 8 NeuronCores.

out[i] = sum_{c,a,b,m} w_e[c,a,b] * ct_e[a,b,m] * f0[i,c,a,m] * f1[i,c,b,m],  e = element_ids[i]

Strategy ("unit-sharded"):
  The contraction decomposes into 568 independent "units": each unit is a
  [128 x 128] block-diagonal weight tile A (4 blocks covering 4 m-values /
  packed (c,m) combos of one element) applied to all atoms of that element.
    el0/el2 (n=32,lm=25): per c: 6 units (m 4q..4q+3) + 8 c-packed m=24 units
    el1     (n=24,lm=16): per c: 4 units, packed densely into 96 partitions
    el3     (n=16,lm=9) : per c: 1 unit (blocks = 2 m's stacked) + 4 c-packed
                          m=8 units + 4 zero dummy units
  Each core owns 71 units (25 el0 + 16 el1 + 25 el2 + 5 el3) and streams ALL
  atoms of the unit's element through them: per 512-atom chunk, one
  K=M<=128 matmul (G = A^T f1 -> PSUM), one DVE multiply h = f0*G (bf16),
  and one ones-matmul (K<=128, M=1) accumulating sum_p h[p,i] into a
  per-(element,chunk) PSUM row (row 32*(chunk%4) of bank chunk//4, which also
  rotates the PE column strips so consecutive acc matmuls overlap).
  Weight loads amortize over ~2050 atoms; per-element chunk counts avoid
  ragged tails; the per-unit PE stream is emitted G*, h*, acc* to keep the
  tensor engine dense. Host precomputes A = w x ct, sorts atoms by element,
  packs f0/f1 as bf16 in the exact SBUF partition layout, sums the partial
  outputs over cores, and scatters back to original atom order.
"""

import math
import os

import numpy as np
import ml_dtypes

BF16 = ml_dtypes.bfloat16
SPECS = [(32, 25), (24, 16), (32, 25), (16, 9)]
N_ATOMS, C, NMAX, LMMAX, E = 8192, 32, 32, 25, 4
NCORES = 8
MAXCH = 5                        # out params sized for up to 5 chunks
UNITS_PER_ELEM = [200, 128, 200, 40]   # el3 includes 4 zero dummy units
SEG_LEN = [u // NCORES for u in UNITS_PER_ELEM]  # per-core: [25, 16, 25, 5]
ELEM_ORDER = [3, 1, 0, 2]  # smallest A-slice first: PE starts sooner


def _chunks_for(count):
    """Split an element's atom count into <=512 matmul chunks (PSUM bank
    limit), last chunk padded to even."""
    nch = max(1, -(-count // 512))
    rem = max(2, count - 512 * (nch - 1))
    rem += rem % 2
    return [512] * (nch - 1) + [rem]


def _unit_descs():
    """Global unit list in order; desc = (e, kind, c_or_t, q)."""
    units = []
    for e in range(E):
        if e in (0, 2):
            for c in range(C):
                for q in range(6):
                    units.append((e, "m", c, q))
            for t in range(8):
                units.append((e, "p", t, 0))
        elif e == 1:
            for c in range(C):
                for q in range(4):
                    units.append((e, "m", c, q))
        else:
            for c in range(C):
                units.append((e, "m", c, 0))
            for t in range(4):
                units.append((e, "p", t, 0))
            for t in range(4):
                units.append((e, "d", t, 0))  # dummy
    assert len(units) == sum(UNITS_PER_ELEM)
    return units


def _unit_blocks(desc, ws_ct):
    """Full [128, 128] lhsT (row=K/b side, col=M/a side) for one unit.
    el1 uses a dense 96x96 region (4 blocks of 24)."""
    e, kind, idx, q = desc
    n, lm = SPECS[e]
    Ae = ws_ct[e]
    out = np.zeros((128, 128), np.float32)
    if kind == "d":
        return out
    if e == 1:
        for j in range(4):
            out[24 * j:24 * j + 24, 24 * j:24 * j + 24] = Ae[idx, :, :, 4 * q + j].T
        return out
    tile = np.zeros((4, 32, 32), np.float32)
    if kind == "m" and e in (0, 2):
        for j in range(4):
            tile[j, :n, :n] = Ae[idx, :, :, 4 * q + j].T
    elif kind == "p" and e in (0, 2):
        for j in range(4):
            tile[j, :, :] = Ae[4 * idx + j, :, :, 24].T
    elif kind == "m":  # e3 main
        for j in range(4):
            for s in range(2):
                tile[j, 16 * s:16 * s + 16, 16 * s:16 * s + 16] = \
                    Ae[idx, :, :, 2 * j + s].T
    else:  # e3 packed m=8
        for j in range(4):
            for s in range(2):
                c = 8 * idx + 2 * j + s
                tile[j, 16 * s:16 * s + 16, 16 * s:16 * s + 16] = Ae[c, :, :, 8].T
    for j in range(4):
        out[32 * j:32 * j + 32, 32 * j:32 * j + 32] = tile[j]
    return out


def _pack_f_element(f, idx, e, npad):
    """Pack one tensor for element e into per-unit-kind arrays.

    Returns dict: 'm' -> [n_main_units, 128(or 4x24 for el1), npad],
                  'p' -> [n_packed, 128, npad] (el0/el2/el3 only)."""
    n, lm = SPECS[e]
    k = len(idx)
    x = np.zeros((npad, C, n, lm), np.float32)
    x[:k] = f[idx][:, :, :n, :lm]
    out = {}
    if e in (0, 2):
        y = x[:, :, :, :24].reshape(npad, C, 32, 6, 4)       # slot,c,b,q,j
        # main units ordered (c, q): [c, q, j, b, slot] -> [c*6+q, 128, npad]
        out["m"] = np.ascontiguousarray(
            y.transpose(1, 3, 4, 2, 0).reshape(C * 6, 128, npad))
        z = x[:, :, :, 24].reshape(npad, 8, 4, 32)           # slot,t,j,b
        out["p"] = np.ascontiguousarray(
            z.transpose(1, 2, 3, 0).reshape(8, 128, npad))
    elif e == 1:
        # dense 96-partition layout: row = 24*j + b
        y = x.reshape(npad, C, 24, 4, 4)                     # slot,c,b,q,j
        out["m"] = np.ascontiguousarray(
            y.transpose(1, 3, 4, 2, 0).reshape(C * 4, 96, npad))
    else:
        y = x[:, :, :, :8].reshape(npad, C, 16, 4, 2)        # slot,c,b,j,s
        out["m"] = np.ascontiguousarray(
            y.transpose(1, 3, 4, 2, 0).reshape(C, 128, npad))
        z = x[:, :, :, 8].reshape(npad, 4, 4, 2, 16)         # slot,t,j,s,b
        out["p"] = np.ascontiguousarray(
            z.transpose(1, 2, 3, 4, 0).reshape(4, 128, npad))
    return out


# ------------------------------------------------------------ bass builder --
def _build_nc(chunks_e):
    import concourse.bacc as bacc
    import concourse.tile as tile
    from concourse import mybir
    from contextlib import ExitStack

    nc = bacc.Bacc(None, target_bir_lowering=False)
    bf = mybir.dt.bfloat16
    f32 = mybir.dt.float32
    UPC = sum(SEG_LEN)  # units per core = 71
    npads = [sum(ch) for ch in chunks_e]

    params = {}
    fshape = {0: [SEG_LEN[0], 128, npads[0]], 1: [SEG_LEN[1], 96, npads[1]],
              2: [SEG_LEN[2], 128, npads[2]], 3: [SEG_LEN[3], 128, npads[3]]}
    for t in ("f0", "f1"):
        for e in range(E):
            name = f"{t}_e{e}"
            params[name] = nc.declare_dram_parameter(name, fshape[e], bf, isOutput=False)
    params["A"] = nc.declare_dram_parameter("A", [128, 128 * UPC], bf, isOutput=False)
    outs = [nc.declare_dram_parameter(f"out{e}", [MAXCH, 512], f32,
                                      isOutput=True) for e in range(E)]

    with tile.TileContext(nc) as tc, ExitStack() as ctx:
        pA = ctx.enter_context(tc.tile_pool(name="A", bufs=1))
        pconst = ctx.enter_context(tc.tile_pool(name="const", bufs=1))
        pf0 = ctx.enter_context(tc.tile_pool(name="f0", bufs=3))
        pf1 = ctx.enter_context(tc.tile_pool(name="f1", bufs=3))
        pg = ctx.enter_context(tc.tile_pool(name="g", bufs=3, space="PSUM"))
        pacc = ctx.enter_context(tc.tile_pool(name="acc", bufs=1, space="PSUM"))
        ph = ctx.enter_context(tc.tile_pool(name="h", bufs=6))
        pst = ctx.enter_context(tc.tile_pool(name="stage", bufs=2))

        A_sb = pA.tile([128, 128 * UPC], bf)
        # per-segment A slices: the first segment's weights land quickly so
        # the PE can start ~12us earlier than with one monolithic DMA
        acol = 0
        for e in ELEM_ORDER:
            w = 128 * SEG_LEN[e]
            nc.sync.dma_start(A_sb[:, acol:acol + w], params["A"][:, acol:acol + w])
            acol += w
        ones = pconst.tile([128, 1], bf)
        nc.any.memset(ones[:, :], 1.0)

        ucol = 0  # column offset into A_sb, advances per unit
        for e in ELEM_ORDER:
            seg = SEG_LEN[e]
            chunks = chunks_e[e]
            npad = npads[e]
            P = 96 if e == 1 else 128  # partition extent (el1 is dense 96)
            nbanks = -(-len(chunks) // 4)
            # chunk pairs (even-aligned): G/h tiles span 2 chunks so equal
            # 512/512 pairs fuse into one DVE multiply
            pairs = [(c0, min(2, len(chunks) - c0)) for c0 in range(0, len(chunks), 2)]
            accs = [pacc.tile([128, 512], f32, name=f"acc{bi}", tag=f"acc{bi}")
                    for bi in range(nbanks)]
            for u in range(seg):
                f0t = pf0.tile([P, npad], bf, tag="tf0")
                f1t = pf1.tile([P, npad], bf, tag="tf1")
                nc.sync.dma_start(f0t[:, :], params[f"f0_e{e}"][u, :, :])
                nc.sync.dma_start(f1t[:, :], params[f"f1_e{e}"][u, :, :])
                # all G matmuls, then all h multiplies, then all acc matmuls:
                # keeps the PE stream dense (HAM stays warm) and lets DVE run
                # one chunk behind the PE
                gt, ht = {}, {}
                off = 0
                for c0, take in pairs:
                    g2 = pg.tile([128, 2, 512], f32, name=f"g{c0}", tag="g")
                    gt[c0] = g2
                    for s in range(take):
                        cn = chunks[c0 + s]
                        nc.tensor.matmul(
                            g2[0:P, s, 0:cn],
                            lhsT=A_sb[0:P, 128 * ucol:128 * ucol + P],
                            rhs=f1t[:, off:off + cn],
                            start=True, stop=True)
                        off += cn
                off = 0
                for c0, take in pairs:
                    h2 = ph.tile([128, 2, 512], bf, name=f"h{c0}", tag="h")
                    ht[c0] = h2
                    if take == 2 and chunks[c0] == 512 and chunks[c0 + 1] == 512:
                        nc.vector.tensor_mul(h2[0:P, :, :],
                                             f0t[:, off:off + 1024],
                                             gt[c0][0:P, :, :])
                        off += 1024
                    else:
                        for s in range(take):
                            cn = chunks[c0 + s]
                            nc.vector.tensor_mul(h2[0:P, s, 0:cn],
                                                 f0t[:, off:off + cn],
                                                 gt[c0][0:P, s, 0:cn])
                            off += cn
                for ci, cn in enumerate(chunks):
                    row = 32 * (ci % 4)
                    nc.tensor.matmul(
                        accs[ci // 4][row:row + 1, 0:cn],
                        lhsT=ones[0:P, 0:1],
                        rhs=ht[(ci // 2) * 2][0:P, ci % 2, 0:cn],
                        start=(u == 0), stop=(u == seg - 1),
                        tile_position=(0, row))
                ucol += 1
            # evict accumulator rows (idle ScalarE) -> stage -> DRAM
            st = pst.tile([128, 512 * nbanks], f32)
            for ci, cn in enumerate(chunks):
                row = 32 * (ci % 4)
                col = 512 * (ci // 4)
                nc.scalar.copy(st[row:row + 1, col:col + cn],
                               accs[ci // 4][row:row + 1, 0:cn])
                nc.sync.dma_start(outs[e][ci:ci + 1, 0:cn],
                                  st[row:row + 1, col:col + cn])
        assert ucol == UPC
    nc.compile()
    return nc


# ------------------------------------------------------------------ kernel --
def _install_ntff_shim():
    """antenv.axon_hooks is missing from this image; recreate it so
    run_bass_kernel_spmd(trace=True) can capture NTFF profiles under axon.
    Only used when KERNEL_TRACE=1 (local timing runs)."""
    import sys, types
    if "antenv.axon_hooks" in sys.modules:
        return
    import antenv
    mod = types.ModuleType("antenv.axon_hooks")
    mod._hook = None
    mod.set_axon_ntff_profile_hook = lambda h: setattr(mod, "_hook", h)
    mod.get_axon_ntff_profile_hook = lambda: mod._hook
    sys.modules["antenv.axon_hooks"] = mod
    antenv.axon_hooks = mod
    from trn_agent_boot.trn_boot import _ntff_profile_via_ctypes
    hook = _ntff_profile_via_ctypes("/opt/axon/libaxon_pjrt.so")
    if hook is not None:
        mod.set_axon_ntff_profile_hook(hook)


def kernel(**inputs):
    f0 = np.asarray(inputs["f0"], np.float32)
    f1 = np.asarray(inputs["f1"], np.float32)
    eids = np.asarray(inputs["element_ids"])
    ws = [np.asarray(inputs[f"w{e}"], np.float32) for e in range(E)]
    cts = [np.asarray(inputs[f"ct{e}"], np.float32) for e in range(E)]

    order = np.argsort(eids, kind="stable")
    idx_e = [order[eids[order] == e] for e in range(E)]
    counts = [len(x) for x in idx_e]
    chunks_e = [_chunks_for(c) for c in counts]
    assert all(len(ch) <= MAXCH for ch in chunks_e)

    ws_ct = [np.einsum("cab,abm->cabm", ws[e], cts[e]).astype(np.float32)
             for e in range(E)]
    units = _unit_descs()

    # global A: [n_units, 128, 128] block-diagonal, then per-core slices
    A_glob = np.zeros((len(units), 128, 128), np.float32)
    for ui, desc in enumerate(units):
        A_glob[ui] = _unit_blocks(desc, ws_ct)
    A_glob = A_glob.astype(BF16)

    # global packed f arrays per element, concatenated in unit order
    packed = {}
    for t, f in (("f0", f0), ("f1", f1)):
        for e in range(E):
            pk = _pack_f_element(f, idx_e[e], e, sum(chunks_e[e]))
            parts = [pk["m"].astype(BF16)]
            if "p" in pk:
                parts.append(pk["p"].astype(BF16))
            n_have = sum(p.shape[0] for p in parts)
            if n_have < UNITS_PER_ELEM[e]:  # el3 dummy units
                zshape = (UNITS_PER_ELEM[e] - n_have,) + parts[0].shape[1:]
                parts.append(np.zeros(zshape, BF16))
            full = np.concatenate(parts, axis=0) if len(parts) > 1 else parts[0]
            assert full.shape[0] == UNITS_PER_ELEM[e]
            packed[(t, e)] = full

    # per-core in_maps: unit slices. Unit order: element-major, so each core's
    # slice of element e's units is [k*SEG_LEN[e], (k+1)*SEG_LEN[e]).
    UPC = sum(SEG_LEN)
    in_maps = []
    for k in range(NCORES):
        m = {}
        # A slice: this core's units, in program order
        bases = np.concatenate([[0], np.cumsum(UNITS_PER_ELEM)])
        rows = []
        for e in ELEM_ORDER:
            b = bases[e]
            rows.append(A_glob[b + k * SEG_LEN[e]: b + (k + 1) * SEG_LEN[e]])
        A_core = np.concatenate(rows, axis=0)              # [71, 128, 128]
        m["A"] = np.ascontiguousarray(
            A_core.transpose(1, 0, 2).reshape(128, 128 * UPC))
        for t in ("f0", "f1"):
            for e in range(E):
                arr = packed[(t, e)][k * SEG_LEN[e]:(k + 1) * SEG_LEN[e]]
                assert arr.shape[0] == SEG_LEN[e], (t, e, k, arr.shape)
                m[f"{t}_e{e}"] = arr
        in_maps.append(m)

    from concourse.bass_utils import run_bass_kernel_spmd

    nc = _build_nc(chunks_e)
    trace = bool(int(os.environ.get("KERNEL_TRACE", "0")))
    if trace:
        try:
            _install_ntff_shim()
        except Exception:
            pass
    res = run_bass_kernel_spmd(nc, in_maps, core_ids=list(range(NCORES)), trace=trace)
    if trace and res.exec_time_ns is not None:
        print(f"HW exec time: {res.exec_time_ns} ns")

    out = np.zeros((N_ATOMS,), np.float32)
    for e in range(E):
        tot = np.zeros((MAXCH, 512), np.float64)
        for k in range(NCORES):
            tot += res.results[k][f"out{e}"].astype(np.float64)
        vals = np.concatenate([tot[ci, :cn] for ci, cn in enumerate(chunks_e[e])])
        out[idx_e[e]] = vals[:counts[e]].astype(np.float32)
    return out[:, None].astype(np.float32)


# revision 36
# speedup vs baseline: 1.1478x; 1.1478x over previous
"""ACE layer (moe_routing) Trainium2 kernel — 8 NeuronCores.

out[i] = sum_{c,a,b,m} w_e[c,a,b] * ct_e[a,b,m] * f0[i,c,a,m] * f1[i,c,b,m],  e = element_ids[i]

Strategy ("unit-sharded"):
  The contraction decomposes into 568 independent "units": each unit is a
  [128 x 128] block-diagonal weight tile A (4 blocks covering 4 m-values /
  packed (c,m) combos of one element) applied to all atoms of that element.
    el0/el2 (n=32,lm=25): per c: 6 units (m 4q..4q+3) + 8 c-packed m=24 units
    el1     (n=24,lm=16): per c: 4 units, packed densely into 96 partitions
    el3     (n=16,lm=9) : per c: 1 unit (blocks = 2 m's stacked) + 4 c-packed
                          m=8 units + 4 zero dummy units
  Each core owns 71 units (25 el0 + 16 el1 + 25 el2 + 5 el3) and streams ALL
  atoms of the unit's element through them: per 512-atom chunk, one
  K=M<=128 matmul (G = A^T f1 -> PSUM), one DVE multiply h = f0*G (bf16),
  and one ones-matmul (K<=128, M=1) accumulating sum_p h[p,i] into a
  per-(element,chunk) PSUM row (row 32*(chunk%4) of bank chunk//4, which also
  rotates the PE column strips so consecutive acc matmuls overlap).
  Weight loads amortize over ~2050 atoms; per-element chunk counts avoid
  ragged tails; the per-unit PE stream is emitted G*, h*, acc* to keep the
  tensor engine dense. Host precomputes A = w x ct, sorts atoms by element,
  packs f0/f1 as bf16 in the exact SBUF partition layout, sums the partial
  outputs over cores, and scatters back to original atom order.
"""

import math
import os

import numpy as np
import ml_dtypes

BF16 = ml_dtypes.bfloat16
SPECS = [(32, 25), (24, 16), (32, 25), (16, 9)]
N_ATOMS, C, NMAX, LMMAX, E = 8192, 32, 32, 25, 4
NCORES = 8
MAXCH = 5                        # out params sized for up to 5 chunks
UNITS_PER_ELEM = [200, 128, 200, 40]   # el3 includes 4 zero dummy units
SEG_LEN = [u // NCORES for u in UNITS_PER_ELEM]  # per-core: [25, 16, 25, 5]
ELEM_ORDER = [0, 1, 2, 3]


def _chunks_for(count):
    """Split an element's atom count into <=512 matmul chunks (PSUM bank
    limit), last chunk padded to even."""
    nch = max(1, -(-count // 512))
    rem = max(2, count - 512 * (nch - 1))
    rem += rem % 2
    return [512] * (nch - 1) + [rem]


def _unit_descs():
    """Global unit list in order; desc = (e, kind, c_or_t, q)."""
    units = []
    for e in range(E):
        if e in (0, 2):
            for c in range(C):
                for q in range(6):
                    units.append((e, "m", c, q))
            for t in range(8):
                units.append((e, "p", t, 0))
        elif e == 1:
            for c in range(C):
                for q in range(4):
                    units.append((e, "m", c, q))
        else:
            for c in range(C):
                units.append((e, "m", c, 0))
            for t in range(4):
                units.append((e, "p", t, 0))
            for t in range(4):
                units.append((e, "d", t, 0))  # dummy
    assert len(units) == sum(UNITS_PER_ELEM)
    return units


def _unit_blocks(desc, ws_ct):
    """Full [128, 128] lhsT (row=K/b side, col=M/a side) for one unit.
    el1 uses a dense 96x96 region (4 blocks of 24)."""
    e, kind, idx, q = desc
    n, lm = SPECS[e]
    Ae = ws_ct[e]
    out = np.zeros((128, 128), np.float32)
    if kind == "d":
        return out
    if e == 1:
        for j in range(4):
            out[24 * j:24 * j + 24, 24 * j:24 * j + 24] = Ae[idx, :, :, 4 * q + j].T
        return out
    tile = np.zeros((4, 32, 32), np.float32)
    if kind == "m" and e in (0, 2):
        for j in range(4):
            tile[j, :n, :n] = Ae[idx, :, :, 4 * q + j].T
    elif kind == "p" and e in (0, 2):
        for j in range(4):
            tile[j, :, :] = Ae[4 * idx + j, :, :, 24].T
    elif kind == "m":  # e3 main
        for j in range(4):
            for s in range(2):
                tile[j, 16 * s:16 * s + 16, 16 * s:16 * s + 16] = \
                    Ae[idx, :, :, 2 * j + s].T
    else:  # e3 packed m=8
        for j in range(4):
            for s in range(2):
                c = 8 * idx + 2 * j + s
                tile[j, 16 * s:16 * s + 16, 16 * s:16 * s + 16] = Ae[c, :, :, 8].T
    for j in range(4):
        out[32 * j:32 * j + 32, 32 * j:32 * j + 32] = tile[j]
    return out


def _pack_f_element(f, idx, e, npad):
    """Pack one tensor for element e into per-unit-kind arrays.

    Returns dict: 'm' -> [n_main_units, 128(or 4x24 for el1), npad],
                  'p' -> [n_packed, 128, npad] (el0/el2/el3 only)."""
    n, lm = SPECS[e]
    k = len(idx)
    x = np.zeros((npad, C, n, lm), np.float32)
    x[:k] = f[idx][:, :, :n, :lm]
    out = {}
    if e in (0, 2):
        y = x[:, :, :, :24].reshape(npad, C, 32, 6, 4)       # slot,c,b,q,j
        # main units ordered (c, q): [c, q, j, b, slot] -> [c*6+q, 128, npad]
        out["m"] = np.ascontiguousarray(
            y.transpose(1, 3, 4, 2, 0).reshape(C * 6, 128, npad))
        z = x[:, :, :, 24].reshape(npad, 8, 4, 32)           # slot,t,j,b
        out["p"] = np.ascontiguousarray(
            z.transpose(1, 2, 3, 0).reshape(8, 128, npad))
    elif e == 1:
        # dense 96-partition layout: row = 24*j + b
        y = x.reshape(npad, C, 24, 4, 4)                     # slot,c,b,q,j
        out["m"] = np.ascontiguousarray(
            y.transpose(1, 3, 4, 2, 0).reshape(C * 4, 96, npad))
    else:
        y = x[:, :, :, :8].reshape(npad, C, 16, 4, 2)        # slot,c,b,j,s
        out["m"] = np.ascontiguousarray(
            y.transpose(1, 3, 4, 2, 0).reshape(C, 128, npad))
        z = x[:, :, :, 8].reshape(npad, 4, 4, 2, 16)         # slot,t,j,s,b
        out["p"] = np.ascontiguousarray(
            z.transpose(1, 2, 3, 4, 0).reshape(4, 128, npad))
    return out


# ------------------------------------------------------------ bass builder --
def _build_nc(chunks_e):
    import concourse.bacc as bacc
    import concourse.tile as tile
    from concourse import mybir
    from contextlib import ExitStack

    nc = bacc.Bacc(None, target_bir_lowering=False)
    bf = mybir.dt.bfloat16
    f32 = mybir.dt.float32
    UPC = sum(SEG_LEN)  # units per core = 71
    npads = [sum(ch) for ch in chunks_e]

    params = {}
    fshape = {0: [SEG_LEN[0], 128, npads[0]], 1: [SEG_LEN[1], 96, npads[1]],
              2: [SEG_LEN[2], 128, npads[2]], 3: [SEG_LEN[3], 128, npads[3]]}
    for t in ("f0", "f1"):
        for e in range(E):
            name = f"{t}_e{e}"
            params[name] = nc.declare_dram_parameter(name, fshape[e], bf, isOutput=False)
    params["A"] = nc.declare_dram_parameter("A", [128, 128 * UPC], bf, isOutput=False)
    outs = [nc.declare_dram_parameter(f"out{e}", [MAXCH, 512], f32,
                                      isOutput=True) for e in range(E)]

    with tile.TileContext(nc) as tc, ExitStack() as ctx:
        pA = ctx.enter_context(tc.tile_pool(name="A", bufs=1))
        pconst = ctx.enter_context(tc.tile_pool(name="const", bufs=1))
        pf0 = ctx.enter_context(tc.tile_pool(name="f0", bufs=3))
        pf1 = ctx.enter_context(tc.tile_pool(name="f1", bufs=3))
        pg = ctx.enter_context(tc.tile_pool(name="g", bufs=6, space="PSUM"))
        pacc = ctx.enter_context(tc.tile_pool(name="acc", bufs=1, space="PSUM"))
        ph = ctx.enter_context(tc.tile_pool(name="h", bufs=6))
        pst = ctx.enter_context(tc.tile_pool(name="stage", bufs=2))

        A_sb = pA.tile([128, 128 * UPC], bf)
        # per-segment A slices: the first segment's weights land quickly so
        # the PE can start ~12us earlier than with one monolithic DMA
        acol = 0
        for e in ELEM_ORDER:
            w = 128 * SEG_LEN[e]
            nc.sync.dma_start(A_sb[:, acol:acol + w], params["A"][:, acol:acol + w])
            acol += w
        ones = pconst.tile([128, 1], bf)
        nc.any.memset(ones[:, :], 1.0)

        ucol = 0  # column offset into A_sb, advances per unit
        for e in ELEM_ORDER:
            seg = SEG_LEN[e]
            chunks = chunks_e[e]
            npad = npads[e]
            P = 96 if e == 1 else 128  # partition extent (el1 is dense 96)
            nbanks = -(-len(chunks) // 4)
            # chunk pairs (even-aligned): G/h tiles span 2 chunks so equal
            # 512/512 pairs fuse into one DVE multiply
            pairs = [(c0, min(2, len(chunks) - c0)) for c0 in range(0, len(chunks), 2)]
            accs = [pacc.tile([128, 512], f32, name=f"acc{bi}", tag=f"acc{bi}")
                    for bi in range(nbanks)]
            for u in range(seg):
                f0t = pf0.tile([P, npad], bf, tag="tf0")
                f1t = pf1.tile([P, npad], bf, tag="tf1")
                nc.sync.dma_start(f0t[:, :], params[f"f0_e{e}"][u, :, :])
                nc.sync.dma_start(f1t[:, :], params[f"f1_e{e}"][u, :, :])
                # all G matmuls, then all h multiplies, then all acc matmuls:
                # keeps the PE stream dense (HAM stays warm) and lets DVE run
                # one chunk behind the PE
                gs, hs = [], []
                off = 0
                for ci, cn in enumerate(chunks):
                    g = pg.tile([128, 512], f32, name=f"g{ci}", tag="g")
                    nc.tensor.matmul(
                        g[0:P, 0:cn],
                        lhsT=A_sb[0:P, 128 * ucol:128 * ucol + P],
                        rhs=f1t[:, off:off + cn],
                        start=True, stop=True)
                    gs.append(g)
                    off += cn
                off = 0
                for ci, cn in enumerate(chunks):
                    h = ph.tile([128, 512], bf, name=f"h{ci}", tag="h")
                    nc.vector.tensor_mul(h[0:P, 0:cn], f0t[:, off:off + cn],
                                         gs[ci][0:P, 0:cn])
                    hs.append(h)
                    off += cn
                for ci, cn in enumerate(chunks):
                    row = 32 * (ci % 4)
                    nc.tensor.matmul(
                        accs[ci // 4][row:row + 1, 0:cn],
                        lhsT=ones[0:P, 0:1], rhs=hs[ci][0:P, 0:cn],
                        start=(u == 0), stop=(u == seg - 1),
                        tile_position=(0, row))
                ucol += 1
            # evict accumulator rows (idle ScalarE) -> stage -> DRAM
            st = pst.tile([128, 512 * nbanks], f32)
            for ci, cn in enumerate(chunks):
                row = 32 * (ci % 4)
                col = 512 * (ci // 4)
                nc.scalar.copy(st[row:row + 1, col:col + cn],
                               accs[ci // 4][row:row + 1, 0:cn])
                nc.sync.dma_start(outs[e][ci:ci + 1, 0:cn],
                                  st[row:row + 1, col:col + cn])
        assert ucol == UPC
    nc.compile()
    return nc


# ------------------------------------------------------------------ kernel --
def _install_ntff_shim():
    """antenv.axon_hooks is missing from this image; recreate it so
    run_bass_kernel_spmd(trace=True) can capture NTFF profiles under axon.
    Only used when KERNEL_TRACE=1 (local timing runs)."""
    import sys, types
    if "antenv.axon_hooks" in sys.modules:
        return
    import antenv
    mod = types.ModuleType("antenv.axon_hooks")
    mod._hook = None
    mod.set_axon_ntff_profile_hook = lambda h: setattr(mod, "_hook", h)
    mod.get_axon_ntff_profile_hook = lambda: mod._hook
    sys.modules["antenv.axon_hooks"] = mod
    antenv.axon_hooks = mod
    from trn_agent_boot.trn_boot import _ntff_profile_via_ctypes
    hook = _ntff_profile_via_ctypes("/opt/axon/libaxon_pjrt.so")
    if hook is not None:
        mod.set_axon_ntff_profile_hook(hook)


def kernel(**inputs):
    f0 = np.asarray(inputs["f0"], np.float32)
    f1 = np.asarray(inputs["f1"], np.float32)
    eids = np.asarray(inputs["element_ids"])
    ws = [np.asarray(inputs[f"w{e}"], np.float32) for e in range(E)]
    cts = [np.asarray(inputs[f"ct{e}"], np.float32) for e in range(E)]

    order = np.argsort(eids, kind="stable")
    idx_e = [order[eids[order] == e] for e in range(E)]
    counts = [len(x) for x in idx_e]
    chunks_e = [_chunks_for(c) for c in counts]
    assert all(len(ch) <= MAXCH for ch in chunks_e)

    ws_ct = [np.einsum("cab,abm->cabm", ws[e], cts[e]).astype(np.float32)
             for e in range(E)]
    units = _unit_descs()

    # global A: [n_units, 128, 128] block-diagonal, then per-core slices
    A_glob = np.zeros((len(units), 128, 128), np.float32)
    for ui, desc in enumerate(units):
        A_glob[ui] = _unit_blocks(desc, ws_ct)
    A_glob = A_glob.astype(BF16)

    # global packed f arrays per element, concatenated in unit order
    packed = {}
    for t, f in (("f0", f0), ("f1", f1)):
        for e in range(E):
            pk = _pack_f_element(f, idx_e[e], e, sum(chunks_e[e]))
            parts = [pk["m"].astype(BF16)]
            if "p" in pk:
                parts.append(pk["p"].astype(BF16))
            n_have = sum(p.shape[0] for p in parts)
            if n_have < UNITS_PER_ELEM[e]:  # el3 dummy units
                zshape = (UNITS_PER_ELEM[e] - n_have,) + parts[0].shape[1:]
                parts.append(np.zeros(zshape, BF16))
            full = np.concatenate(parts, axis=0) if len(parts) > 1 else parts[0]
            assert full.shape[0] == UNITS_PER_ELEM[e]
            packed[(t, e)] = full

    # per-core in_maps: unit slices. Unit order: element-major, so each core's
    # slice of element e's units is [k*SEG_LEN[e], (k+1)*SEG_LEN[e]).
    UPC = sum(SEG_LEN)
    in_maps = []
    for k in range(NCORES):
        m = {}
        # A slice: this core's units, in program order
        bases = np.concatenate([[0], np.cumsum(UNITS_PER_ELEM)])
        rows = []
        for e in ELEM_ORDER:
            b = bases[e]
            rows.append(A_glob[b + k * SEG_LEN[e]: b + (k + 1) * SEG_LEN[e]])
        A_core = np.concatenate(rows, axis=0)              # [71, 128, 128]
        m["A"] = np.ascontiguousarray(
            A_core.transpose(1, 0, 2).reshape(128, 128 * UPC))
        for t in ("f0", "f1"):
            for e in range(E):
                arr = packed[(t, e)][k * SEG_LEN[e]:(k + 1) * SEG_LEN[e]]
                assert arr.shape[0] == SEG_LEN[e], (t, e, k, arr.shape)
                m[f"{t}_e{e}"] = arr
        in_maps.append(m)

    from concourse.bass_utils import run_bass_kernel_spmd

    nc = _build_nc(chunks_e)
    trace = bool(int(os.environ.get("KERNEL_TRACE", "0")))
    if trace:
        try:
            _install_ntff_shim()
        except Exception:
            pass
    res = run_bass_kernel_spmd(nc, in_maps, core_ids=list(range(NCORES)), trace=trace)
    if trace and res.exec_time_ns is not None:
        print(f"HW exec time: {res.exec_time_ns} ns")

    out = np.zeros((N_ATOMS,), np.float32)
    for e in range(E):
        tot = np.zeros((MAXCH, 512), np.float64)
        for k in range(NCORES):
            tot += res.results[k][f"out{e}"].astype(np.float64)
        vals = np.concatenate([tot[ci, :cn] for ci, cn in enumerate(chunks_e[e])])
        out[idx_e[e]] = vals[:counts[e]].astype(np.float32)
    return out[:, None].astype(np.float32)


# revision 37
# speedup vs baseline: 1.2104x; 1.0545x over previous
"""ACE layer (moe_routing) Trainium2 kernel — 8 NeuronCores.

out[i] = sum_{c,a,b,m} w_e[c,a,b] * ct_e[a,b,m] * f0[i,c,a,m] * f1[i,c,b,m],  e = element_ids[i]

Strategy ("unit-sharded"):
  The contraction decomposes into 568 independent "units": each unit is a
  [128 x 128] block-diagonal weight tile A (4 blocks covering 4 m-values /
  packed (c,m) combos of one element) applied to all atoms of that element.
    el0/el2 (n=32,lm=25): per c: 6 units (m 4q..4q+3) + 8 c-packed m=24 units
    el1     (n=24,lm=16): per c: 4 units, packed densely into 96 partitions
    el3     (n=16,lm=9) : per c: 1 unit (blocks = 2 m's stacked) + 4 c-packed
                          m=8 units + 4 zero dummy units
  Each core owns 71 units (25 el0 + 16 el1 + 25 el2 + 5 el3) and streams ALL
  atoms of the unit's element through them: per 512-atom chunk, one
  K=M<=128 matmul (G = A^T f1 -> PSUM), one DVE multiply h = f0*G (bf16),
  and one ones-matmul (K<=128, M=1) accumulating sum_p h[p,i] into a
  per-(element,chunk) PSUM row (row 32*(chunk%4) of bank chunk//4, which also
  rotates the PE column strips so consecutive acc matmuls overlap).
  Weight loads amortize over ~2050 atoms; per-element chunk counts avoid
  ragged tails; the per-unit PE stream is emitted G*, h*, acc* to keep the
  tensor engine dense. Host precomputes A = w x ct, sorts atoms by element,
  packs f0/f1 as bf16 in the exact SBUF partition layout, sums the partial
  outputs over cores, and scatters back to original atom order.
"""

import math
import os

import numpy as np
import ml_dtypes

BF16 = ml_dtypes.bfloat16
SPECS = [(32, 25), (24, 16), (32, 25), (16, 9)]
N_ATOMS, C, NMAX, LMMAX, E = 8192, 32, 32, 25, 4
NCORES = 8
MAXCH = 5                        # out params sized for up to 5 chunks
UNITS_PER_ELEM = [200, 128, 200, 40]   # el3 includes 4 zero dummy units
SEG_LEN = [u // NCORES for u in UNITS_PER_ELEM]  # per-core: [25, 16, 25, 5]
ELEM_ORDER = [0, 1, 2, 3]


def _chunks_for(count):
    """Split an element's atom count into <=512 matmul chunks (PSUM bank
    limit), last chunk padded to even."""
    nch = max(1, -(-count // 512))
    rem = max(2, count - 512 * (nch - 1))
    rem += rem % 2
    return [512] * (nch - 1) + [rem]


def _unit_descs():
    """Global unit list in order; desc = (e, kind, c_or_t, q)."""
    units = []
    for e in range(E):
        if e in (0, 2):
            for c in range(C):
                for q in range(6):
                    units.append((e, "m", c, q))
            for t in range(8):
                units.append((e, "p", t, 0))
        elif e == 1:
            for c in range(C):
                for q in range(4):
                    units.append((e, "m", c, q))
        else:
            for c in range(C):
                units.append((e, "m", c, 0))
            for t in range(4):
                units.append((e, "p", t, 0))
            for t in range(4):
                units.append((e, "d", t, 0))  # dummy
    assert len(units) == sum(UNITS_PER_ELEM)
    return units


def _unit_blocks(desc, ws_ct):
    """Full [128, 128] lhsT (row=K/b side, col=M/a side) for one unit.
    el1 uses a dense 96x96 region (4 blocks of 24)."""
    e, kind, idx, q = desc
    n, lm = SPECS[e]
    Ae = ws_ct[e]
    out = np.zeros((128, 128), np.float32)
    if kind == "d":
        return out
    if e == 1:
        for j in range(4):
            out[24 * j:24 * j + 24, 24 * j:24 * j + 24] = Ae[idx, :, :, 4 * q + j].T
        return out
    tile = np.zeros((4, 32, 32), np.float32)
    if kind == "m" and e in (0, 2):
        for j in range(4):
            tile[j, :n, :n] = Ae[idx, :, :, 4 * q + j].T
    elif kind == "p" and e in (0, 2):
        for j in range(4):
            tile[j, :, :] = Ae[4 * idx + j, :, :, 24].T
    elif kind == "m":  # e3 main
        for j in range(4):
            for s in range(2):
                tile[j, 16 * s:16 * s + 16, 16 * s:16 * s + 16] = \
                    Ae[idx, :, :, 2 * j + s].T
    else:  # e3 packed m=8
        for j in range(4):
            for s in range(2):
                c = 8 * idx + 2 * j + s
                tile[j, 16 * s:16 * s + 16, 16 * s:16 * s + 16] = Ae[c, :, :, 8].T
    for j in range(4):
        out[32 * j:32 * j + 32, 32 * j:32 * j + 32] = tile[j]
    return out


def _pack_f_element(f, idx, e, npad):
    """Pack one tensor for element e into per-unit-kind arrays.

    Returns dict: 'm' -> [n_main_units, 128(or 4x24 for el1), npad],
                  'p' -> [n_packed, 128, npad] (el0/el2/el3 only)."""
    n, lm = SPECS[e]
    k = len(idx)
    x = np.zeros((npad, C, n, lm), np.float32)
    x[:k] = f[idx][:, :, :n, :lm]
    out = {}
    if e in (0, 2):
        y = x[:, :, :, :24].reshape(npad, C, 32, 6, 4)       # slot,c,b,q,j
        # main units ordered (c, q): [c, q, j, b, slot] -> [c*6+q, 128, npad]
        out["m"] = np.ascontiguousarray(
            y.transpose(1, 3, 4, 2, 0).reshape(C * 6, 128, npad))
        z = x[:, :, :, 24].reshape(npad, 8, 4, 32)           # slot,t,j,b
        out["p"] = np.ascontiguousarray(
            z.transpose(1, 2, 3, 0).reshape(8, 128, npad))
    elif e == 1:
        # dense 96-partition layout: row = 24*j + b
        y = x.reshape(npad, C, 24, 4, 4)                     # slot,c,b,q,j
        out["m"] = np.ascontiguousarray(
            y.transpose(1, 3, 4, 2, 0).reshape(C * 4, 96, npad))
    else:
        y = x[:, :, :, :8].reshape(npad, C, 16, 4, 2)        # slot,c,b,j,s
        out["m"] = np.ascontiguousarray(
            y.transpose(1, 3, 4, 2, 0).reshape(C, 128, npad))
        z = x[:, :, :, 8].reshape(npad, 4, 4, 2, 16)         # slot,t,j,s,b
        out["p"] = np.ascontiguousarray(
            z.transpose(1, 2, 3, 4, 0).reshape(4, 128, npad))
    return out


# ------------------------------------------------------------ bass builder --
def _build_nc(chunks_e):
    import concourse.bacc as bacc
    import concourse.tile as tile
    from concourse import mybir
    from contextlib import ExitStack

    nc = bacc.Bacc(None, target_bir_lowering=False)
    bf = mybir.dt.bfloat16
    f32 = mybir.dt.float32
    UPC = sum(SEG_LEN)  # units per core = 71
    npads = [sum(ch) for ch in chunks_e]

    params = {}
    fshape = {0: [SEG_LEN[0], 128, npads[0]], 1: [SEG_LEN[1], 96, npads[1]],
              2: [SEG_LEN[2], 128, npads[2]], 3: [SEG_LEN[3], 128, npads[3]]}
    for t in ("f0", "f1"):
        for e in range(E):
            name = f"{t}_e{e}"
            params[name] = nc.declare_dram_parameter(name, fshape[e], bf, isOutput=False)
    params["A"] = nc.declare_dram_parameter("A", [128, 128 * UPC], bf, isOutput=False)
    outs = [nc.declare_dram_parameter(f"out{e}", [MAXCH, 512], f32,
                                      isOutput=True) for e in range(E)]

    with tile.TileContext(nc) as tc, ExitStack() as ctx:
        pA = ctx.enter_context(tc.tile_pool(name="A", bufs=1))
        pconst = ctx.enter_context(tc.tile_pool(name="const", bufs=1))
        pf0 = ctx.enter_context(tc.tile_pool(name="f0", bufs=4))
        pf1 = ctx.enter_context(tc.tile_pool(name="f1", bufs=4))
        pg = ctx.enter_context(tc.tile_pool(name="g", bufs=6, space="PSUM"))
        pacc = ctx.enter_context(tc.tile_pool(name="acc", bufs=1, space="PSUM"))
        ph = ctx.enter_context(tc.tile_pool(name="h", bufs=6))
        pst = ctx.enter_context(tc.tile_pool(name="stage", bufs=2))

        A_sb = pA.tile([128, 128 * UPC], bf)
        # per-segment A slices: the first segment's weights land quickly so
        # the PE can start ~12us earlier than with one monolithic DMA
        acol = 0
        for e in ELEM_ORDER:
            w = 128 * SEG_LEN[e]
            nc.sync.dma_start(A_sb[:, acol:acol + w], params["A"][:, acol:acol + w])
            acol += w
        ones = pconst.tile([128, 1], bf)
        nc.any.memset(ones[:, :], 1.0)

        ucol = 0  # column offset into A_sb, advances per unit
        for e in ELEM_ORDER:
            seg = SEG_LEN[e]
            chunks = chunks_e[e]
            npad = npads[e]
            P = 96 if e == 1 else 128  # partition extent (el1 is dense 96)
            nbanks = -(-len(chunks) // 4)
            # chunk pairs (even-aligned): G/h tiles span 2 chunks so equal
            # 512/512 pairs fuse into one DVE multiply
            pairs = [(c0, min(2, len(chunks) - c0)) for c0 in range(0, len(chunks), 2)]
            accs = [pacc.tile([128, 512], f32, name=f"acc{bi}", tag=f"acc{bi}")
                    for bi in range(nbanks)]
            for u in range(seg):
                f0t = pf0.tile([P, npad], bf, tag="tf0")
                f1t = pf1.tile([P, npad], bf, tag="tf1")
                nc.sync.dma_start(f0t[:, :], params[f"f0_e{e}"][u, :, :])
                nc.sync.dma_start(f1t[:, :], params[f"f1_e{e}"][u, :, :])
                # all G matmuls, then all h multiplies, then all acc matmuls:
                # keeps the PE stream dense (HAM stays warm) and lets DVE run
                # one chunk behind the PE
                gs, hs = [], []
                off = 0
                for ci, cn in enumerate(chunks):
                    g = pg.tile([128, 512], f32, name=f"g{ci}", tag="g")
                    nc.tensor.matmul(
                        g[0:P, 0:cn],
                        lhsT=A_sb[0:P, 128 * ucol:128 * ucol + P],
                        rhs=f1t[:, off:off + cn],
                        start=True, stop=True)
                    gs.append(g)
                    off += cn
                off = 0
                for ci, cn in enumerate(chunks):
                    h = ph.tile([128, 512], bf, name=f"h{ci}", tag="h")
                    nc.vector.tensor_mul(h[0:P, 0:cn], f0t[:, off:off + cn],
                                         gs[ci][0:P, 0:cn])
                    hs.append(h)
                    off += cn
                for ci, cn in enumerate(chunks):
                    row = 32 * (ci % 4)
                    nc.tensor.matmul(
                        accs[ci // 4][row:row + 1, 0:cn],
                        lhsT=ones[0:P, 0:1], rhs=hs[ci][0:P, 0:cn],
                        start=(u == 0), stop=(u == seg - 1),
                        tile_position=(0, row))
                ucol += 1
            # evict accumulator rows (idle ScalarE) -> stage -> DRAM
            st = pst.tile([128, 512 * nbanks], f32)
            for ci, cn in enumerate(chunks):
                row = 32 * (ci % 4)
                col = 512 * (ci // 4)
                nc.scalar.copy(st[row:row + 1, col:col + cn],
                               accs[ci // 4][row:row + 1, 0:cn])
                nc.sync.dma_start(outs[e][ci:ci + 1, 0:cn],
                                  st[row:row + 1, col:col + cn])
        assert ucol == UPC
    nc.compile()
    return nc


# ------------------------------------------------------------------ kernel --
def _install_ntff_shim():
    """antenv.axon_hooks is missing from this image; recreate it so
    run_bass_kernel_spmd(trace=True) can capture NTFF profiles under axon.
    Only used when KERNEL_TRACE=1 (local timing runs)."""
    import sys, types
    if "antenv.axon_hooks" in sys.modules:
        return
    import antenv
    mod = types.ModuleType("antenv.axon_hooks")
    mod._hook = None
    mod.set_axon_ntff_profile_hook = lambda h: setattr(mod, "_hook", h)
    mod.get_axon_ntff_profile_hook = lambda: mod._hook
    sys.modules["antenv.axon_hooks"] = mod
    antenv.axon_hooks = mod
    from trn_agent_boot.trn_boot import _ntff_profile_via_ctypes
    hook = _ntff_profile_via_ctypes("/opt/axon/libaxon_pjrt.so")
    if hook is not None:
        mod.set_axon_ntff_profile_hook(hook)


def kernel(**inputs):
    f0 = np.asarray(inputs["f0"], np.float32)
    f1 = np.asarray(inputs["f1"], np.float32)
    eids = np.asarray(inputs["element_ids"])
    ws = [np.asarray(inputs[f"w{e}"], np.float32) for e in range(E)]
    cts = [np.asarray(inputs[f"ct{e}"], np.float32) for e in range(E)]

    order = np.argsort(eids, kind="stable")
    idx_e = [order[eids[order] == e] for e in range(E)]
    counts = [len(x) for x in idx_e]
    chunks_e = [_chunks_for(c) for c in counts]
    assert all(len(ch) <= MAXCH for ch in chunks_e)

    ws_ct = [np.einsum("cab,abm->cabm", ws[e], cts[e]).astype(np.float32)
             for e in range(E)]
    units = _unit_descs()

    # global A: [n_units, 128, 128] block-diagonal, then per-core slices
    A_glob = np.zeros((len(units), 128, 128), np.float32)
    for ui, desc in enumerate(units):
        A_glob[ui] = _unit_blocks(desc, ws_ct)
    A_glob = A_glob.astype(BF16)

    # global packed f arrays per element, concatenated in unit order
    packed = {}
    for t, f in (("f0", f0), ("f1", f1)):
        for e in range(E):
            pk = _pack_f_element(f, idx_e[e], e, sum(chunks_e[e]))
            parts = [pk["m"].astype(BF16)]
            if "p" in pk:
                parts.append(pk["p"].astype(BF16))
            n_have = sum(p.shape[0] for p in parts)
            if n_have < UNITS_PER_ELEM[e]:  # el3 dummy units
                zshape = (UNITS_PER_ELEM[e] - n_have,) + parts[0].shape[1:]
                parts.append(np.zeros(zshape, BF16))
            full = np.concatenate(parts, axis=0) if len(parts) > 1 else parts[0]
            assert full.shape[0] == UNITS_PER_ELEM[e]
            packed[(t, e)] = full

    # per-core in_maps: unit slices. Unit order: element-major, so each core's
    # slice of element e's units is [k*SEG_LEN[e], (k+1)*SEG_LEN[e]).
    UPC = sum(SEG_LEN)
    in_maps = []
    for k in range(NCORES):
        m = {}
        # A slice: this core's units, in program order
        bases = np.concatenate([[0], np.cumsum(UNITS_PER_ELEM)])
        rows = []
        for e in ELEM_ORDER:
            b = bases[e]
            rows.append(A_glob[b + k * SEG_LEN[e]: b + (k + 1) * SEG_LEN[e]])
        A_core = np.concatenate(rows, axis=0)              # [71, 128, 128]
        m["A"] = np.ascontiguousarray(
            A_core.transpose(1, 0, 2).reshape(128, 128 * UPC))
        for t in ("f0", "f1"):
            for e in range(E):
                arr = packed[(t, e)][k * SEG_LEN[e]:(k + 1) * SEG_LEN[e]]
                assert arr.shape[0] == SEG_LEN[e], (t, e, k, arr.shape)
                m[f"{t}_e{e}"] = arr
        in_maps.append(m)

    from concourse.bass_utils import run_bass_kernel_spmd

    nc = _build_nc(chunks_e)
    trace = bool(int(os.environ.get("KERNEL_TRACE", "0")))
    if trace:
        try:
            _install_ntff_shim()
        except Exception:
            pass
    res = run_bass_kernel_spmd(nc, in_maps, core_ids=list(range(NCORES)), trace=trace)
    if trace and res.exec_time_ns is not None:
        print(f"HW exec time: {res.exec_time_ns} ns")

    out = np.zeros((N_ATOMS,), np.float32)
    for e in range(E):
        tot = np.zeros((MAXCH, 512), np.float64)
        for k in range(NCORES):
            tot += res.results[k][f"out{e}"].astype(np.float64)
        vals = np.concatenate([tot[ci, :cn] for ci, cn in enumerate(chunks_e[e])])
        out[idx_e[e]] = vals[:counts[e]].astype(np.float32)
    return out[:, None].astype(np.float32)
